# revision 18
# baseline (speedup 1.0000x reference)
"""Trainium2 Bass kernel for nn_Network_63763084476816 (GNN message passing).

The batched graph is structurally fixed: per graph, 38 clinical + 36 pixel
nodes, self-edges everywhere, and a complete bipartite pixel<->clinical edge
set.  Mean aggregation therefore collapses to dense math:

    h_c = relu(x_c @ (W_self + W_msg/37) + S_pix @ (W_msg/37) + b_g)
    h_p = relu(x_p @ (W_self + W_msg/39) + S_clin @ (W_msg/39) + b_g)
    gap = mean_p h_p
    out = relu([h_c | gap] @ W1 + b1) @ W2 + b2

Sharding: pure data parallel, 128 graphs per core on 8 cores; weights
(including W1) replicated.  Embeddings ship feature-major ([FV, node*BC+b]).

v3 notes (all measured on HW):
- All big tensors bf16 (PSUM accumulation fp32; rel err ~4e-3 vs 2e-2 gate).
- The DMA fabric is 16 shared engines x ~26 GB/s, one packet per
  partition-row per descriptor, and each queue dispatches ~1 packet/28ns.
  So descriptors use LONG rows (>=9 KB) and the two HWDGE queues split the
  stream: sync gets [gw|v|pixel] + the W1 tail, scalar gets clinical + the
  W1 head + the fp32 tail params.
- b_g is folded into the node sums on the host (v = (W_msg/deg)^-T b_g,
  exact for the generator's b_g=0), so the relu drain needs no bias and
  splits across ACT and DVE - the PSUM banks recycle at PE pace instead of
  single-engine pace.
- The PE p-state ramps (1.2 GHz until ~3us of continuous execution, then
  2.4 GHz) and resets on idle gaps, so a chain of dummy matmuls warms the
  PE from the preamble until real work arrives.
- gap tree-folds run per-group on the otherwise idle GpSimd engine.
- The [128,1] result is PE-transposed (x identity) to one partition so the
  output DMA is a single contiguous 512B descriptor.
"""

import sys

for _p in ("/opt/trn_rl_repo",):
    if _p not in sys.path:
        sys.path.insert(0, _p)

import numpy as np

_B = 1024
_NCORES = 8
_BC = _B // _NCORES  # 128 graphs per core
_NCLIN = 38
_NPIX = 36
_FV = 128
_HID = 512
_NCHUNK = 39  # K-chunks of 128 in the 4992-wide MLP contraction
_CCOLS = _NCLIN * _BC  # 4864
_PCOLS = _NPIX * _BC  # 4608
_S4P0 = 512  # [gw(512) | s4pix(512) | s4clin(512) | clinical | pixel]
_S4C0 = 1024
_CLIN0 = 1536
_PIX0 = _CLIN0 + _CCOLS  # 6400
_XTW = _PIX0 + _PCOLS  # 11008

_W1GROUPS = [14, 14, 11]  # scalar ring, sync ring, sync ring
_NWARM = 22  # PE warmup matmuls

_CACHE = {}


def _build_bass():
    import concourse.bacc as bacc
    import concourse.mybir as mybir
    import concourse.tile as tile

    f32 = mybir.dt.float32
    bf16 = mybir.dt.bfloat16
    relu = mybir.ActivationFunctionType.Relu

    nc = bacc.Bacc("TRN2", target_bir_lowering=False, debug=False,
                   num_devices=_NCORES)

    xt_d = nc.dram_tensor("xt", [_FV, _XTW], bf16, kind="ExternalInput")
    w1_d = nc.dram_tensor("w1", [_FV, _NCHUNK * _HID], bf16, kind="ExternalInput")
    aux_d = nc.dram_tensor("aux", [_BC, _HID + 1 + _FV], f32, kind="ExternalInput")
    rowaux_d = nc.dram_tensor("rowaux", [1, _HID + _BC], bf16, kind="ExternalInput")
    out_d = nc.dram_tensor("out", [1, _BC], f32, kind="ExternalOutput")

    _LOWP = "bf16 operands by design; matmul accumulation stays fp32"

    with tile.TileContext(nc) as tc:
        with tc.tile_pool(name="main", bufs=1) as pool, \
             tc.tile_pool(name="hps", bufs=3, space="PSUM") as pps, \
             tc.tile_pool(name="zps", bufs=1, space="PSUM") as ppz, \
             tc.tile_pool(name="tps", bufs=1, space="PSUM") as ppt:

            xt = pool.tile([_FV, _XTW], bf16, name="xt", tag="xt")
            w1sb = [
                pool.tile([_FV, gch, _HID], bf16, name=f"w1sb{g}", tag=f"w1sb{g}")
                for g, gch in enumerate(_W1GROUPS)
            ]
            auxsb = pool.tile([_BC, _HID + 1 + _FV], f32, name="auxsb", tag="auxsb")
            rowsb = pool.tile([1, _HID + _BC], bf16, name="rowsb", tag="rowsb")

            def w1_dma(engine, g):
                c0 = sum(_W1GROUPS[:g])
                engine.dma_start(
                    w1sb[g][:],
                    w1_d.ap()[:, c0 * _HID:(c0 + _W1GROUPS[g]) * _HID].rearrange(
                        "p (c n) -> p c n", c=_W1GROUPS[g]),
                )

            # Embeddings first on BOTH queues (they gate the whole h phase;
            # W1 is consumed much later), each section split across the two
            # queues so it lands at combined-fabric rate. Clinical (plus the
            # host-computed node sums) leads since it gates h_c; pixel
            # follows; W1 after that; the fp32 tail params go last.
            _CH = _CLIN0 + _CCOLS // 2  # clinical halfway
            _PH = _PIX0 + _PCOLS // 2  # pixel halfway
            nc.sync.dma_start(xt[:, :_CH], xt_d.ap()[:, :_CH])
            nc.scalar.dma_start(xt[:, _CH:_PIX0], xt_d.ap()[:, _CH:_PIX0])
            nc.sync.dma_start(xt[:, _PIX0:_PH], xt_d.ap()[:, _PIX0:_PH])
            nc.scalar.dma_start(xt[:, _PH:], xt_d.ap()[:, _PH:])
            w1_dma(nc.sync, 1)
            w1_dma(nc.scalar, 0)
            w1_dma(nc.sync, 2)
            nc.scalar.dma_start(auxsb[:], aux_d.ap())
            nc.scalar.dma_start(rowsb[:], rowaux_d.ap())

            # PE warmup: dummy matmuls on a memset tile keep the PE
            # continuously executing (and ramped to full clock) until the
            # first real matmul's operands land.
            wmu = pool.tile([_FV, _HID], bf16, name="wmu", tag="wmu")
            nc.gpsimd.memset(wmu[:], 1.0)
            wps = ppt.tile([_FV, _HID], f32, name="wps", tag="tps")
            for _ in range(_NWARM):
                nc.tensor.matmul(wps[:], wmu[:, :_FV], wmu[:],
                                 start=True, stop=True)

            # ---- per-graph node sums: packed bf16 pairwise tree-folds ----
            scr = pool.tile([_FV, 5120], bf16, name="scr", tag="scr")

            def fold_blocks(eng, src_ap, nblk, dst_ap, base):
                """Pairwise-fold nblk 128-col blocks of src_ap into the
                single 128-col block dst_ap, using scr[:, base:] as scratch."""
                cur, n, off = src_ap, nblk, base
                while True:
                    h = n // 2
                    w = h * _BC
                    if h == 1 and n % 2 == 0:
                        eng.tensor_add(dst_ap, cur[:, :w], cur[:, w:2 * w])
                        return
                    nxt = scr[:, off:off + w + (n % 2) * _BC]
                    eng.tensor_add(nxt[:, :w], cur[:, :w], cur[:, w:2 * w])
                    if n % 2:
                        eng.tensor_copy(nxt[:, w:w + _BC], cur[:, 2 * w:2 * w + _BC])
                    cur, n, off = nxt, h + (n % 2), off + w + (n % 2) * _BC

            combT = pool.tile([_FV, _NCHUNK * _BC], bf16, name="combT", tag="combT")
            hpT = pool.tile([_FV, _PCOLS], bf16, name="hpT", tag="hpT")

            def h_phase(nblk, a_ap, wm_ap, s4_ap, src0, dest, psname, di0):
                # Two 4-block groups share one 2-bank PSUM tile and drain in
                # a single wide relu op, alternating ACT/DVE, so the banks
                # recycle at PE production pace.
                g0, gi = 0, 0
                while g0 < nblk:
                    c1 = min(4, nblk - g0)
                    c2 = min(4, nblk - g0 - c1)
                    wa, wb = c1 * _BC, c2 * _BC
                    ps = pps.tile([_FV, 1024], f32, name=f"{psname}{gi}", tag="hps")
                    nc.tensor.matmul(
                        ps[:, :wa], a_ap,
                        xt[:, src0 + g0 * _BC: src0 + (g0 + c1) * _BC],
                        start=True, stop=False,
                    )
                    nc.tensor.matmul(ps[:, :wa], wm_ap, s4_ap[:, :wa],
                                     start=False, stop=True)
                    if c2:
                        nc.tensor.matmul(
                            ps[:, 512:512 + wb], a_ap,
                            xt[:, src0 + (g0 + c1) * _BC: src0 + (g0 + c1 + c2) * _BC],
                            start=True, stop=False,
                        )
                        nc.tensor.matmul(ps[:, 512:512 + wb], wm_ap, s4_ap[:, :wb],
                                         start=False, stop=True)
                    dst = dest[:, g0 * _BC: (g0 + c1 + c2) * _BC]
                    src = ps[:, :512 + wb] if c2 else ps[:, :wa]
                    with nc.allow_low_precision(reason=_LOWP):
                        if (gi + di0) % 2 == 0:
                            nc.scalar.activation(dst, src, relu)
                        else:
                            nc.vector.tensor_scalar_max(dst, src, 0.0)
                    g0 += c1 + c2
                    gi += 1

            # h^T tiles: clinical into combT blocks 0..37, pixel into hpT.
            h_phase(_NCLIN, xt[:, 0:_FV], xt[:, 2 * _FV:3 * _FV],
                    xt[:, _S4P0:_S4C0], _CLIN0, combT, "psc", 0)
            h_phase(_NPIX, xt[:, _FV:2 * _FV], xt[:, 3 * _FV:4 * _FV],
                    xt[:, _S4C0:_CLIN0], _PIX0, hpT, "psp", 1)

            # gap block: packed DVE tree-fold right behind the h_p drains.
            # It lands long before the MLP's PE stream reaches chunk 38.
            # (The 1/36 is folded into W1's last rows on the host.)
            fold_blocks(nc.vector, hpT[:], _NPIX, combT[:, _NCLIN * _BC:], 0)

            # MLP layer 1: psz[b, n] = sum_k combined[b, k] W1[k, n] (+ b1).
            psz = ppz.tile([_BC, _HID], f32, name="psz", tag="psz")

            def mlp_chunk(k, start, stop):
                goff = 0
                for g, gch in enumerate(_W1GROUPS):
                    if k < goff + gch:
                        nc.tensor.matmul(
                            psz[:],
                            combT[:, k * _BC:(k + 1) * _BC],
                            w1sb[g][:, k - goff, :],
                            start=start, stop=stop,
                        )
                        return
                    goff += gch

            for k in range(_NCHUNK - 1):
                mlp_chunk(k, start=(k == 0), stop=False)
            nc.tensor.matmul(psz[:], rowsb[:, _HID:], rowsb[:, :_HID],
                             start=False, stop=False)  # + b1
            mlp_chunk(_NCHUNK - 1, start=False, stop=True)  # gap chunk last

            # MLP layer 2 fused: one DVE op does relu (max with 0), the W2
            # multiply, and the free-dim sum, reading psz from PSUM.
            # (tensor_tensor_reduce wedges the device on this path;
            # scalar_tensor_tensor with accum_out is HW-verified.)
            zw = pool.tile([_BC, _HID], f32, name="zw", tag="zw")
            osum = pool.tile([_BC, 1], f32, name="osum", tag="osum")
            nc.vector.scalar_tensor_tensor(
                out=zw[:], in0=psz[:], scalar=0.0, in1=auxsb[:, :_HID],
                op0=mybir.AluOpType.max, op1=mybir.AluOpType.mult,
                accum_out=osum[:],
            )
            ofin = pool.tile([_BC, 1], f32, name="ofin", tag="ofin")
            nc.vector.tensor_scalar_add(ofin[:], osum[:], auxsb[:, _HID:_HID + 1])
            # Transpose [128,1] -> [1,128] on the PE (ofin^T @ I) so the
            # output DMA is one contiguous 512B descriptor.
            pst = ppt.tile([1, _BC], f32, name="pst", tag="tps")
            nc.tensor.matmul(pst[:], ofin[:], auxsb[:, _HID + 1:],
                             start=True, stop=True)
            osb = pool.tile([1, _BC], f32, name="osb", tag="osb")
            nc.vector.tensor_copy(osb[:], pst[:])
            nc.sync.dma_start(out_d.ap(), osb[:])

    nc.compile()
    return nc


def _host_prep(W_self, W_msg, b_g, W1, b1, W2, b2):
    import ml_dtypes

    f32 = np.float32
    bf = ml_dtypes.bfloat16
    wmc = np.asarray(W_msg, f32) / f32(37.0)
    wmp = np.asarray(W_msg, f32) / f32(39.0)
    ws = np.asarray(W_self, f32)
    bg = np.asarray(b_g, f32).reshape(-1)
    # Fold b_g into the node sums: (W_msg/deg)^T (S + v) = (W_msg/deg)^T S
    # + b_g with v = (W_msg/deg)^-T b_g (exact for the generator's b_g=0).
    v_c = np.linalg.solve(wmc.T, bg).astype(f32)
    v_p = np.linalg.solve(wmp.T, bg).astype(f32)
    smalls = np.empty((_FV, _S4P0), dtype=f32)
    smalls[:, 0:_FV] = ws + wmc
    smalls[:, _FV:2 * _FV] = ws + wmp
    smalls[:, 2 * _FV:3 * _FV] = wmc
    smalls[:, 3 * _FV:4 * _FV] = wmp
    w1m = np.array(W1, dtype=f32, copy=True)
    w1m[_NCLIN * _FV:, :] /= f32(_NPIX)
    # Pack to SBUF layout [p, (chunk, n)]: w1p[p, c*HID+n] = w1m[c*FV+p, n].
    w1m = np.ascontiguousarray(
        w1m.reshape(_NCHUNK, _FV, _HID).transpose(1, 0, 2).reshape(_FV, -1).astype(bf))
    aux = np.zeros((_BC, _HID + 1 + _FV), dtype=f32)
    aux[:, :_HID] = np.asarray(W2, f32).reshape(1, _HID)
    aux[:, _HID] = f32(np.asarray(b2, f32).reshape(-1)[0])
    aux[:, _HID + 1:] = np.eye(_FV, dtype=f32)
    rowaux = np.empty((1, _HID + _BC), dtype=f32)
    rowaux[0, :_HID] = np.asarray(b1, f32)
    rowaux[0, _HID:] = f32(1.0)
    return smalls.astype(bf), v_c, v_p, w1m, aux, rowaux.astype(bf)


def _xt_for_core(clinical, image, k, smalls, v_c, v_p):
    import ml_dtypes

    bf = ml_dtypes.bfloat16
    sl = slice(k * _BC, (k + 1) * _BC)
    xc = np.ascontiguousarray(clinical[sl].transpose(2, 1, 0)).reshape(_FV, _CCOLS)
    xp = np.ascontiguousarray(image[sl].transpose(2, 1, 0)).reshape(_FV, _PCOLS)
    # Host-side message-passing node sums (+ the b_g fold), replicated x4
    # for the N=512 aggregate matmuls.
    s_pix = xp.reshape(_FV, _NPIX, _BC).sum(axis=1) + v_c[:, None]
    s_clin = xc.reshape(_FV, _NCLIN, _BC).sum(axis=1) + v_p[:, None]
    out = np.empty((_FV, _XTW), dtype=bf)
    out[:, :_S4P0] = smalls
    out[:, _S4P0:_S4C0] = np.tile(s_pix, (1, 4)).astype(bf)
    out[:, _S4C0:_CLIN0] = np.tile(s_clin, (1, 4)).astype(bf)
    out[:, _CLIN0:_PIX0] = xc.astype(bf)
    out[:, _PIX0:] = xp.astype(bf)
    return out


def kernel(**inputs):
    clinical = np.asarray(inputs["clinical_embeddings"], np.float32)
    image = np.asarray(inputs["image_embeddings"], np.float32)
    smalls, v_c, v_p, w1m, aux, rowaux = _host_prep(
        inputs["W_self"], inputs["W_msg"], inputs["b_g"],
        inputs["W1"], inputs["b1"], inputs["W2"], inputs["b2"],
    )

    if "nc" not in _CACHE:
        _CACHE["nc"] = _build_bass()
    nc = _CACHE["nc"]

    in_maps = [
        {
            "xt": _xt_for_core(clinical, image, k, smalls, v_c, v_p),
            "w1": w1m,
            "aux": aux,
            "rowaux": rowaux,
        }
        for k in range(_NCORES)
    ]

    from concourse.bass_utils import run_bass_kernel_spmd

    res = run_bass_kernel_spmd(
        nc, in_maps, core_ids=list(range(_NCORES)),
        trace=bool(_CACHE.get("trace", False)),
        **_CACHE.get("run_kwargs", {}),
    )
    _CACHE["last_results"] = res
    out = np.concatenate(
        [np.asarray(r["out"], np.float32).reshape(_BC, 1) for r in res.results],
        axis=0)
    return np.ascontiguousarray(out)


# revision 20
# speedup vs baseline: 1.2076x; 1.2076x over previous
"""Trainium2 Bass kernel for nn_Network_63763084476816 (GNN message passing).

The batched graph is structurally fixed: per graph, 38 clinical + 36 pixel
nodes, self-edges everywhere, and a complete bipartite pixel<->clinical edge
set.  Mean aggregation therefore collapses to dense math:

    h_c = relu(x_c @ (W_self + W_msg/37) + S_pix @ (W_msg/37) + b_g)
    h_p = relu(x_p @ (W_self + W_msg/39) + S_clin @ (W_msg/39) + b_g)
    gap = mean_p h_p
    out = relu([h_c | gap] @ W1 + b1) @ W2 + b2

Sharding: pure data parallel, 128 graphs per core on 8 cores; weights
(including W1) replicated.  Embeddings ship feature-major ([FV, node*BC+b]).

v3 notes (all measured on HW):
- All big tensors bf16 (PSUM accumulation fp32; rel err ~4e-3 vs 2e-2 gate).
- The DMA fabric is 16 shared engines x ~26 GB/s, one packet per
  partition-row per descriptor, and each queue dispatches ~1 packet/28ns.
  So descriptors use LONG rows (>=9 KB) and the two HWDGE queues split the
  stream: sync gets [gw|v|pixel] + the W1 tail, scalar gets clinical + the
  W1 head + the fp32 tail params.
- b_g is folded into the node sums on the host (v = (W_msg/deg)^-T b_g,
  exact for the generator's b_g=0), so the relu drain needs no bias and
  splits across ACT and DVE - the PSUM banks recycle at PE pace instead of
  single-engine pace.
- The PE p-state ramps (1.2 GHz until ~3us of continuous execution, then
  2.4 GHz) and resets on idle gaps, so a chain of dummy matmuls warms the
  PE from the preamble until real work arrives.
- gap tree-folds run per-group on the otherwise idle GpSimd engine.
- The [128,1] result is PE-transposed (x identity) to one partition so the
  output DMA is a single contiguous 512B descriptor.
"""

import sys

for _p in ("/opt/trn_rl_repo",):
    if _p not in sys.path:
        sys.path.insert(0, _p)

import numpy as np

_B = 1024
_NCORES = 8
_BC = _B // _NCORES  # 128 graphs per core
_NCLIN = 38
_NPIX = 36
_FV = 128
_HID = 512
_NCHUNK = 39  # K-chunks of 128 in the 4992-wide MLP contraction
_CCOLS = _NCLIN * _BC  # 4864
_PCOLS = _NPIX * _BC  # 4608
_S4P0 = 512  # [gw(512) | s4pix(512) | s4clin(512) | clinical | pixel]
_S4C0 = 1024
_CLIN0 = 1536
_PIX0 = _CLIN0 + _CCOLS  # 6400
_XTW = _PIX0 + _PCOLS  # 11008

_W1GROUPS = [14, 14, 11]  # scalar ring, sync ring, sync ring
_NWARM = 22  # PE warmup matmuls

_CACHE = {}


def _build_bass():
    import concourse.bacc as bacc
    import concourse.mybir as mybir
    import concourse.tile as tile

    f32 = mybir.dt.float32
    bf16 = mybir.dt.bfloat16
    relu = mybir.ActivationFunctionType.Relu

    nc = bacc.Bacc("TRN2", target_bir_lowering=False, debug=False,
                   num_devices=_NCORES)

    xt_d = nc.dram_tensor("xt", [_FV, _XTW], bf16, kind="ExternalInput")
    w1_d = nc.dram_tensor("w1", [_FV, _NCHUNK * _HID], bf16, kind="ExternalInput")
    aux_d = nc.dram_tensor("aux", [_BC, _HID + 1 + _FV], f32, kind="ExternalInput")
    rowaux_d = nc.dram_tensor("rowaux", [1, _HID + _BC], bf16, kind="ExternalInput")
    out_d = nc.dram_tensor("out", [1, _BC], f32, kind="ExternalOutput")

    _LOWP = "bf16 operands by design; matmul accumulation stays fp32"

    with tile.TileContext(nc) as tc:
        with tc.tile_pool(name="main", bufs=1) as pool, \
             tc.tile_pool(name="hps", bufs=3, space="PSUM") as pps, \
             tc.tile_pool(name="zps", bufs=1, space="PSUM") as ppz, \
             tc.tile_pool(name="tps", bufs=1, space="PSUM") as ppt:

            xt = pool.tile([_FV, _XTW], bf16, name="xt", tag="xt")
            w1sb = [
                pool.tile([_FV, gch, _HID], bf16, name=f"w1sb{g}", tag=f"w1sb{g}")
                for g, gch in enumerate(_W1GROUPS)
            ]
            auxsb = pool.tile([_BC, _HID + 1 + _FV], f32, name="auxsb", tag="auxsb")
            rowsb = pool.tile([1, _HID + _BC], bf16, name="rowsb", tag="rowsb")

            def w1_dma(engine, g):
                c0 = sum(_W1GROUPS[:g])
                engine.dma_start(
                    w1sb[g][:],
                    w1_d.ap()[:, c0 * _HID:(c0 + _W1GROUPS[g]) * _HID].rearrange(
                        "p (c n) -> p c n", c=_W1GROUPS[g]),
                )

            # Embeddings first on BOTH queues (they gate the whole h phase;
            # W1 is consumed much later), each section split across the two
            # queues so it lands at combined-fabric rate. Clinical (plus the
            # host-computed node sums) leads since it gates h_c; pixel
            # follows; W1 after that; the fp32 tail params go last.
            # Byte-balanced queues: sync = [smalls|s4|clinA] + W1 groups 1,2;
            # scalar = clinB + pixel + W1 group 0 + fp32 tail params. The
            # MLP consumes W1 chunks in arrival order (see below).
            _CH = _CLIN0 + _CCOLS // 2  # clinical halfway
            nc.sync.dma_start(xt[:, :_CH], xt_d.ap()[:, :_CH])
            nc.scalar.dma_start(xt[:, _CH:_PIX0], xt_d.ap()[:, _CH:_PIX0])
            nc.scalar.dma_start(xt[:, _PIX0:], xt_d.ap()[:, _PIX0:])
            w1_dma(nc.sync, 1)
            w1_dma(nc.scalar, 0)
            w1_dma(nc.sync, 2)
            nc.scalar.dma_start(auxsb[:], aux_d.ap())
            nc.scalar.dma_start(rowsb[:], rowaux_d.ap())

            # PE warmup: dummy matmuls on a memset tile keep the PE
            # continuously executing (and ramped to full clock) until the
            # first real matmul's operands land.
            wmu = pool.tile([_FV, _HID], bf16, name="wmu", tag="wmu")
            nc.gpsimd.memset(wmu[:], 1.0)
            wps = ppt.tile([_FV, _HID], f32, name="wps", tag="tps")
            for _ in range(_NWARM):
                nc.tensor.matmul(wps[:], wmu[:, :_FV], wmu[:],
                                 start=True, stop=True)

            # ---- per-graph node sums: packed bf16 pairwise tree-folds ----
            scr = pool.tile([_FV, 5120], bf16, name="scr", tag="scr")

            def fold_blocks(eng, src_ap, nblk, dst_ap, base):
                """Pairwise-fold nblk 128-col blocks of src_ap into the
                single 128-col block dst_ap, using scr[:, base:] as scratch."""
                cur, n, off = src_ap, nblk, base
                while True:
                    h = n // 2
                    w = h * _BC
                    if h == 1 and n % 2 == 0:
                        eng.tensor_add(dst_ap, cur[:, :w], cur[:, w:2 * w])
                        return
                    nxt = scr[:, off:off + w + (n % 2) * _BC]
                    eng.tensor_add(nxt[:, :w], cur[:, :w], cur[:, w:2 * w])
                    if n % 2:
                        eng.tensor_copy(nxt[:, w:w + _BC], cur[:, 2 * w:2 * w + _BC])
                    cur, n, off = nxt, h + (n % 2), off + w + (n % 2) * _BC

            combT = pool.tile([_FV, _NCHUNK * _BC], bf16, name="combT", tag="combT")
            hpT = pool.tile([_FV, _PCOLS], bf16, name="hpT", tag="hpT")

            def h_phase(nblk, a_ap, wm_ap, s4_ap, src0, dest, psname, di0):
                # Two 4-block groups share one 2-bank PSUM tile and drain in
                # a single wide relu op, alternating ACT/DVE, so the banks
                # recycle at PE production pace.
                g0, gi = 0, 0
                while g0 < nblk:
                    c1 = min(4, nblk - g0)
                    c2 = min(4, nblk - g0 - c1)
                    wa, wb = c1 * _BC, c2 * _BC
                    ps = pps.tile([_FV, 1024], f32, name=f"{psname}{gi}", tag="hps")
                    nc.tensor.matmul(
                        ps[:, :wa], a_ap,
                        xt[:, src0 + g0 * _BC: src0 + (g0 + c1) * _BC],
                        start=True, stop=False,
                    )
                    nc.tensor.matmul(ps[:, :wa], wm_ap, s4_ap[:, :wa],
                                     start=False, stop=True)
                    if c2:
                        nc.tensor.matmul(
                            ps[:, 512:512 + wb], a_ap,
                            xt[:, src0 + (g0 + c1) * _BC: src0 + (g0 + c1 + c2) * _BC],
                            start=True, stop=False,
                        )
                        nc.tensor.matmul(ps[:, 512:512 + wb], wm_ap, s4_ap[:, :wb],
                                         start=False, stop=True)
                    dst = dest[:, g0 * _BC: (g0 + c1 + c2) * _BC]
                    src = ps[:, :512 + wb] if c2 else ps[:, :wa]
                    with nc.allow_low_precision(reason=_LOWP):
                        if (gi + di0) % 2 == 0:
                            nc.scalar.activation(dst, src, relu)
                        else:
                            nc.vector.tensor_scalar_max(dst, src, 0.0)
                    g0 += c1 + c2
                    gi += 1

            # h^T tiles: clinical into combT blocks 0..37, pixel into hpT.
            h_phase(_NCLIN, xt[:, 0:_FV], xt[:, 2 * _FV:3 * _FV],
                    xt[:, _S4P0:_S4C0], _CLIN0, combT, "psc", 0)
            h_phase(_NPIX, xt[:, _FV:2 * _FV], xt[:, 3 * _FV:4 * _FV],
                    xt[:, _S4C0:_CLIN0], _PIX0, hpT, "psp", 1)

            # gap block: packed DVE tree-fold right behind the h_p drains.
            # It lands long before the MLP's PE stream reaches chunk 38.
            # (The 1/36 is folded into W1's last rows on the host.)
            fold_blocks(nc.vector, hpT[:], _NPIX, combT[:, _NCLIN * _BC:], 0)

            # MLP layer 1: psz[b, n] = sum_k combined[b, k] W1[k, n] (+ b1).
            psz = ppz.tile([_BC, _HID], f32, name="psz", tag="psz")

            def mlp_chunk(k, start, stop):
                goff = 0
                for g, gch in enumerate(_W1GROUPS):
                    if k < goff + gch:
                        nc.tensor.matmul(
                            psz[:],
                            combT[:, k * _BC:(k + 1) * _BC],
                            w1sb[g][:, k - goff, :],
                            start=start, stop=stop,
                        )
                        return
                    goff += gch

            # Consume chunks in W1 *arrival* order: group 1 (sync, lands
            # first), group 0 (scalar), then the sync tail; the gap chunk
            # (38) stays last. PSUM accumulation order is free.
            order = list(range(14, 28)) + list(range(0, 14)) + list(range(28, 38))
            for i, k in enumerate(order):
                mlp_chunk(k, start=(i == 0), stop=False)
            nc.tensor.matmul(psz[:], rowsb[:, _HID:], rowsb[:, :_HID],
                             start=False, stop=False)  # + b1
            mlp_chunk(_NCHUNK - 1, start=False, stop=True)  # gap chunk last

            # MLP layer 2 fused: one DVE op does relu (max with 0), the W2
            # multiply, and the free-dim sum, reading psz from PSUM.
            # (tensor_tensor_reduce wedges the device on this path;
            # scalar_tensor_tensor with accum_out is HW-verified.)
            zw = pool.tile([_BC, _HID], f32, name="zw", tag="zw")
            osum = pool.tile([_BC, 1], f32, name="osum", tag="osum")
            nc.vector.scalar_tensor_tensor(
                out=zw[:], in0=psz[:], scalar=0.0, in1=auxsb[:, :_HID],
                op0=mybir.AluOpType.max, op1=mybir.AluOpType.mult,
                accum_out=osum[:],
            )
            ofin = pool.tile([_BC, 1], f32, name="ofin", tag="ofin")
            nc.vector.tensor_scalar_add(ofin[:], osum[:], auxsb[:, _HID:_HID + 1])
            # Transpose [128,1] -> [1,128] on the PE (ofin^T @ I) so the
            # output DMA is one contiguous 512B descriptor.
            pst = ppt.tile([1, _BC], f32, name="pst", tag="tps")
            nc.tensor.matmul(pst[:], ofin[:], auxsb[:, _HID + 1:],
                             start=True, stop=True)
            osb = pool.tile([1, _BC], f32, name="osb", tag="osb")
            nc.vector.tensor_copy(osb[:], pst[:])
            nc.sync.dma_start(out_d.ap(), osb[:])

    nc.compile()
    return nc


def _host_prep(W_self, W_msg, b_g, W1, b1, W2, b2):
    import ml_dtypes

    f32 = np.float32
    bf = ml_dtypes.bfloat16
    wmc = np.asarray(W_msg, f32) / f32(37.0)
    wmp = np.asarray(W_msg, f32) / f32(39.0)
    ws = np.asarray(W_self, f32)
    bg = np.asarray(b_g, f32).reshape(-1)
    # Fold b_g into the node sums: (W_msg/deg)^T (S + v) = (W_msg/deg)^T S
    # + b_g with v = (W_msg/deg)^-T b_g (exact for the generator's b_g=0).
    v_c = np.linalg.solve(wmc.T, bg).astype(f32)
    v_p = np.linalg.solve(wmp.T, bg).astype(f32)
    smalls = np.empty((_FV, _S4P0), dtype=f32)
    smalls[:, 0:_FV] = ws + wmc
    smalls[:, _FV:2 * _FV] = ws + wmp
    smalls[:, 2 * _FV:3 * _FV] = wmc
    smalls[:, 3 * _FV:4 * _FV] = wmp
    w1m = np.array(W1, dtype=f32, copy=True)
    w1m[_NCLIN * _FV:, :] /= f32(_NPIX)
    # Pack to SBUF layout [p, (chunk, n)]: w1p[p, c*HID+n] = w1m[c*FV+p, n].
    w1m = np.ascontiguousarray(
        w1m.reshape(_NCHUNK, _FV, _HID).transpose(1, 0, 2).reshape(_FV, -1).astype(bf))
    aux = np.zeros((_BC, _HID + 1 + _FV), dtype=f32)
    aux[:, :_HID] = np.asarray(W2, f32).reshape(1, _HID)
    aux[:, _HID] = f32(np.asarray(b2, f32).reshape(-1)[0])
    aux[:, _HID + 1:] = np.eye(_FV, dtype=f32)
    rowaux = np.empty((1, _HID + _BC), dtype=f32)
    rowaux[0, :_HID] = np.asarray(b1, f32)
    rowaux[0, _HID:] = f32(1.0)
    return smalls.astype(bf), v_c, v_p, w1m, aux, rowaux.astype(bf)


def _xt_for_core(clinical, image, k, smalls, v_c, v_p):
    import ml_dtypes

    bf = ml_dtypes.bfloat16
    sl = slice(k * _BC, (k + 1) * _BC)
    xc = np.ascontiguousarray(clinical[sl].transpose(2, 1, 0)).reshape(_FV, _CCOLS)
    xp = np.ascontiguousarray(image[sl].transpose(2, 1, 0)).reshape(_FV, _PCOLS)
    # Host-side message-passing node sums (+ the b_g fold), replicated x4
    # for the N=512 aggregate matmuls.
    s_pix = xp.reshape(_FV, _NPIX, _BC).sum(axis=1) + v_c[:, None]
    s_clin = xc.reshape(_FV, _NCLIN, _BC).sum(axis=1) + v_p[:, None]
    out = np.empty((_FV, _XTW), dtype=bf)
    out[:, :_S4P0] = smalls
    out[:, _S4P0:_S4C0] = np.tile(s_pix, (1, 4)).astype(bf)
    out[:, _S4C0:_CLIN0] = np.tile(s_clin, (1, 4)).astype(bf)
    out[:, _CLIN0:_PIX0] = xc.astype(bf)
    out[:, _PIX0:] = xp.astype(bf)
    return out


def kernel(**inputs):
    clinical = np.asarray(inputs["clinical_embeddings"], np.float32)
    image = np.asarray(inputs["image_embeddings"], np.float32)
    smalls, v_c, v_p, w1m, aux, rowaux = _host_prep(
        inputs["W_self"], inputs["W_msg"], inputs["b_g"],
        inputs["W1"], inputs["b1"], inputs["W2"], inputs["b2"],
    )

    if "nc" not in _CACHE:
        _CACHE["nc"] = _build_bass()
    nc = _CACHE["nc"]

    in_maps = [
        {
            "xt": _xt_for_core(clinical, image, k, smalls, v_c, v_p),
            "w1": w1m,
            "aux": aux,
            "rowaux": rowaux,
        }
        for k in range(_NCORES)
    ]

    from concourse.bass_utils import run_bass_kernel_spmd

    res = run_bass_kernel_spmd(
        nc, in_maps, core_ids=list(range(_NCORES)),
        trace=bool(_CACHE.get("trace", False)),
        **_CACHE.get("run_kwargs", {}),
    )
    _CACHE["last_results"] = res
    out = np.concatenate(
        [np.asarray(r["out"], np.float32).reshape(_BC, 1) for r in res.results],
        axis=0)
    return np.ascontiguousarray(out)


# revision 22
# speedup vs baseline: 1.2836x; 1.0629x over previous
"""Trainium2 Bass kernel for nn_Network_63763084476816 (GNN message passing).

The batched graph is structurally fixed: per graph, 38 clinical + 36 pixel
nodes, self-edges everywhere, and a complete bipartite pixel<->clinical edge
set.  Mean aggregation therefore collapses to dense math:

    h_c = relu(x_c @ (W_self + W_msg/37) + S_pix @ (W_msg/37) + b_g)
    h_p = relu(x_p @ (W_self + W_msg/39) + S_clin @ (W_msg/39) + b_g)
    gap = mean_p h_p
    out = relu([h_c | gap] @ W1 + b1) @ W2 + b2

Sharding: pure data parallel, 128 graphs per core on 8 cores; weights
(including W1) replicated.  Embeddings ship feature-major ([FV, node*BC+b]).

v3 notes (all measured on HW):
- All big tensors bf16 (PSUM accumulation fp32; rel err ~4e-3 vs 2e-2 gate).
- The DMA fabric is 16 shared engines x ~26 GB/s, one packet per
  partition-row per descriptor, and each queue dispatches ~1 packet/28ns.
  So descriptors use LONG rows (>=9 KB) and the two HWDGE queues split the
  stream: sync gets [gw|v|pixel] + the W1 tail, scalar gets clinical + the
  W1 head + the fp32 tail params.
- b_g is folded into the node sums on the host (v = (W_msg/deg)^-T b_g,
  exact for the generator's b_g=0), so the relu drain needs no bias and
  splits across ACT and DVE - the PSUM banks recycle at PE pace instead of
  single-engine pace.
- The PE p-state ramps (1.2 GHz until ~3us of continuous execution, then
  2.4 GHz) and resets on idle gaps, so a chain of dummy matmuls warms the
  PE from the preamble until real work arrives.
- gap tree-folds run per-group on the otherwise idle GpSimd engine.
- The [128,1] result is PE-transposed (x identity) to one partition so the
  output DMA is a single contiguous 512B descriptor.
"""

import sys

for _p in ("/opt/trn_rl_repo",):
    if _p not in sys.path:
        sys.path.insert(0, _p)

import numpy as np

_B = 1024
_NCORES = 8
_BC = _B // _NCORES  # 128 graphs per core
_NCLIN = 38
_NPIX = 36
_FV = 128
_HID = 512
_NCHUNK = 39  # K-chunks of 128 in the 4992-wide MLP contraction
_CCOLS = _NCLIN * _BC  # 4864
_PCOLS = _NPIX * _BC  # 4608
_S4P0 = 512  # [gw(512) | s4pix(512) | s4clin(512) | clinical | pixel]
_S4C0 = 1024
_CLIN0 = 1536
_PIX0 = _CLIN0 + _CCOLS  # 6400
_XTW = _PIX0 + _PCOLS  # 11008

_W1GROUPS = [7, 7, 7, 7, 7, 4]  # groups 0-1: scalar ring; 2-5: sync ring
_NWARM = 22  # PE warmup matmuls

_CACHE = {}


def _build_bass():
    import concourse.bacc as bacc
    import concourse.mybir as mybir
    import concourse.tile as tile

    f32 = mybir.dt.float32
    bf16 = mybir.dt.bfloat16
    relu = mybir.ActivationFunctionType.Relu

    nc = bacc.Bacc("TRN2", target_bir_lowering=False, debug=False,
                   num_devices=_NCORES)

    xt_d = nc.dram_tensor("xt", [_FV, _XTW], bf16, kind="ExternalInput")
    w1_d = nc.dram_tensor("w1", [_FV, _NCHUNK * _HID], bf16, kind="ExternalInput")
    aux_d = nc.dram_tensor("aux", [_BC, _HID + 1 + _FV], f32, kind="ExternalInput")
    rowaux_d = nc.dram_tensor("rowaux", [1, _HID + _BC], bf16, kind="ExternalInput")
    out_d = nc.dram_tensor("out", [1, _BC], f32, kind="ExternalOutput")

    _LOWP = "bf16 operands by design; matmul accumulation stays fp32"

    with tile.TileContext(nc) as tc:
        with tc.tile_pool(name="main", bufs=1) as pool, \
             tc.tile_pool(name="hps", bufs=3, space="PSUM") as pps, \
             tc.tile_pool(name="zps", bufs=1, space="PSUM") as ppz, \
             tc.tile_pool(name="tps", bufs=1, space="PSUM") as ppt:

            xt = pool.tile([_FV, _XTW], bf16, name="xt", tag="xt")
            w1sb = [
                pool.tile([_FV, gch, _HID], bf16, name=f"w1sb{g}", tag=f"w1sb{g}")
                for g, gch in enumerate(_W1GROUPS)
            ]
            auxsb = pool.tile([_BC, _HID + 1 + _FV], f32, name="auxsb", tag="auxsb")
            rowsb = pool.tile([1, _HID + _BC], bf16, name="rowsb", tag="rowsb")

            def w1_dma(engine, g):
                c0 = sum(_W1GROUPS[:g])
                engine.dma_start(
                    w1sb[g][:],
                    w1_d.ap()[:, c0 * _HID:(c0 + _W1GROUPS[g]) * _HID].rearrange(
                        "p (c n) -> p c n", c=_W1GROUPS[g]),
                )

            # Embeddings first on BOTH queues (they gate the whole h phase;
            # W1 is consumed much later), each section split across the two
            # queues so it lands at combined-fabric rate. Clinical (plus the
            # host-computed node sums) leads since it gates h_c; pixel
            # follows; W1 after that; the fp32 tail params go last.
            # Byte-balanced queues: sync = [smalls|s4|clinA] + W1 groups 1,2;
            # scalar = clinB + pixel + W1 group 0 + fp32 tail params. The
            # MLP consumes W1 chunks in arrival order (see below).
            _CH = _CLIN0 + _CCOLS // 2  # clinical halfway
            nc.sync.dma_start(xt[:, :_CH], xt_d.ap()[:, :_CH])
            nc.scalar.dma_start(xt[:, _CH:_PIX0], xt_d.ap()[:, _CH:_PIX0])
            nc.scalar.dma_start(xt[:, _PIX0:], xt_d.ap()[:, _PIX0:])
            w1_dma(nc.sync, 2)
            w1_dma(nc.scalar, 0)
            w1_dma(nc.sync, 3)
            w1_dma(nc.scalar, 1)
            w1_dma(nc.sync, 4)
            w1_dma(nc.sync, 5)
            nc.scalar.dma_start(auxsb[:], aux_d.ap())
            nc.scalar.dma_start(rowsb[:], rowaux_d.ap())

            # PE warmup: dummy matmuls on a memset tile keep the PE
            # continuously executing (and ramped to full clock) until the
            # first real matmul's operands land.
            wmu = pool.tile([_FV, _HID], bf16, name="wmu", tag="wmu")
            nc.gpsimd.memset(wmu[:], 1.0)
            wps = ppt.tile([_FV, _HID], f32, name="wps", tag="tps")
            for _ in range(_NWARM):
                nc.tensor.matmul(wps[:], wmu[:, :_FV], wmu[:],
                                 start=True, stop=True)

            # ---- per-graph node sums: packed bf16 pairwise tree-folds ----
            scr = pool.tile([_FV, 5120], bf16, name="scr", tag="scr")

            def fold_blocks(eng, src_ap, nblk, dst_ap, base):
                """Pairwise-fold nblk 128-col blocks of src_ap into the
                single 128-col block dst_ap, using scr[:, base:] as scratch."""
                cur, n, off = src_ap, nblk, base
                while True:
                    h = n // 2
                    w = h * _BC
                    if h == 1 and n % 2 == 0:
                        eng.tensor_add(dst_ap, cur[:, :w], cur[:, w:2 * w])
                        return
                    nxt = scr[:, off:off + w + (n % 2) * _BC]
                    eng.tensor_add(nxt[:, :w], cur[:, :w], cur[:, w:2 * w])
                    if n % 2:
                        eng.tensor_copy(nxt[:, w:w + _BC], cur[:, 2 * w:2 * w + _BC])
                    cur, n, off = nxt, h + (n % 2), off + w + (n % 2) * _BC

            combT = pool.tile([_FV, _NCHUNK * _BC], bf16, name="combT", tag="combT")
            hpT = pool.tile([_FV, _PCOLS], bf16, name="hpT", tag="hpT")

            def h_phase(nblk, a_ap, wm_ap, s4_ap, src0, dest, psname, di0):
                # Two 4-block groups share one 2-bank PSUM tile and drain in
                # a single wide relu op, alternating ACT/DVE, so the banks
                # recycle at PE production pace.
                g0, gi = 0, 0
                while g0 < nblk:
                    c1 = min(4, nblk - g0)
                    c2 = min(4, nblk - g0 - c1)
                    wa, wb = c1 * _BC, c2 * _BC
                    ps = pps.tile([_FV, 1024], f32, name=f"{psname}{gi}", tag="hps")
                    nc.tensor.matmul(
                        ps[:, :wa], a_ap,
                        xt[:, src0 + g0 * _BC: src0 + (g0 + c1) * _BC],
                        start=True, stop=False,
                    )
                    nc.tensor.matmul(ps[:, :wa], wm_ap, s4_ap[:, :wa],
                                     start=False, stop=True)
                    if c2:
                        nc.tensor.matmul(
                            ps[:, 512:512 + wb], a_ap,
                            xt[:, src0 + (g0 + c1) * _BC: src0 + (g0 + c1 + c2) * _BC],
                            start=True, stop=False,
                        )
                        nc.tensor.matmul(ps[:, 512:512 + wb], wm_ap, s4_ap[:, :wb],
                                         start=False, stop=True)
                    dst = dest[:, g0 * _BC: (g0 + c1 + c2) * _BC]
                    src = ps[:, :512 + wb] if c2 else ps[:, :wa]
                    with nc.allow_low_precision(reason=_LOWP):
                        if (gi + di0) % 2 == 0:
                            nc.scalar.activation(dst, src, relu)
                        else:
                            nc.vector.tensor_scalar_max(dst, src, 0.0)
                    g0 += c1 + c2
                    gi += 1

            # h^T tiles: clinical into combT blocks 0..37, pixel into hpT.
            h_phase(_NCLIN, xt[:, 0:_FV], xt[:, 2 * _FV:3 * _FV],
                    xt[:, _S4P0:_S4C0], _CLIN0, combT, "psc", 0)
            h_phase(_NPIX, xt[:, _FV:2 * _FV], xt[:, 3 * _FV:4 * _FV],
                    xt[:, _S4C0:_CLIN0], _PIX0, hpT, "psp", 1)

            # gap block: packed DVE tree-fold right behind the h_p drains.
            # It lands long before the MLP's PE stream reaches chunk 38.
            # (The 1/36 is folded into W1's last rows on the host.)
            fold_blocks(nc.vector, hpT[:], _NPIX, combT[:, _NCLIN * _BC:], 0)

            # MLP layer 1: psz[b, n] = sum_k combined[b, k] W1[k, n] (+ b1).
            psz = ppz.tile([_BC, _HID], f32, name="psz", tag="psz")

            def mlp_chunk(k, start, stop):
                goff = 0
                for g, gch in enumerate(_W1GROUPS):
                    if k < goff + gch:
                        nc.tensor.matmul(
                            psz[:],
                            combT[:, k * _BC:(k + 1) * _BC],
                            w1sb[g][:, k - goff, :],
                            start=start, stop=stop,
                        )
                        return
                    goff += gch

            # Consume chunks in W1 *arrival* order: group 1 (sync, lands
            # first), group 0 (scalar), then the sync tail; the gap chunk
            # (38) stays last. PSUM accumulation order is free.
            order = (list(range(14, 21)) + list(range(0, 7))
                     + list(range(21, 28)) + list(range(7, 14))
                     + list(range(28, 38)))
            for i, k in enumerate(order):
                mlp_chunk(k, start=(i == 0), stop=False)
            nc.tensor.matmul(psz[:], rowsb[:, _HID:], rowsb[:, :_HID],
                             start=False, stop=False)  # + b1
            mlp_chunk(_NCHUNK - 1, start=False, stop=True)  # gap chunk last

            # MLP layer 2 fused: one DVE op does relu (max with 0), the W2
            # multiply, and the free-dim sum, reading psz from PSUM.
            # (tensor_tensor_reduce wedges the device on this path;
            # scalar_tensor_tensor with accum_out is HW-verified.)
            zw = pool.tile([_BC, _HID], f32, name="zw", tag="zw")
            osum = pool.tile([_BC, 1], f32, name="osum", tag="osum")
            nc.vector.scalar_tensor_tensor(
                out=zw[:], in0=psz[:], scalar=0.0, in1=auxsb[:, :_HID],
                op0=mybir.AluOpType.max, op1=mybir.AluOpType.mult,
                accum_out=osum[:],
            )
            ofin = pool.tile([_BC, 1], f32, name="ofin", tag="ofin")
            nc.vector.tensor_scalar_add(ofin[:], osum[:], auxsb[:, _HID:_HID + 1])
            # Transpose [128,1] -> [1,128] on the PE (ofin^T @ I) so the
            # output DMA is one contiguous 512B descriptor.
            pst = ppt.tile([1, _BC], f32, name="pst", tag="tps")
            nc.tensor.matmul(pst[:], ofin[:], auxsb[:, _HID + 1:],
                             start=True, stop=True)
            osb = pool.tile([1, _BC], f32, name="osb", tag="osb")
            nc.vector.tensor_copy(osb[:], pst[:])
            nc.sync.dma_start(out_d.ap(), osb[:])

    nc.compile()
    return nc


def _host_prep(W_self, W_msg, b_g, W1, b1, W2, b2):
    import ml_dtypes

    f32 = np.float32
    bf = ml_dtypes.bfloat16
    wmc = np.asarray(W_msg, f32) / f32(37.0)
    wmp = np.asarray(W_msg, f32) / f32(39.0)
    ws = np.asarray(W_self, f32)
    bg = np.asarray(b_g, f32).reshape(-1)
    # Fold b_g into the node sums: (W_msg/deg)^T (S + v) = (W_msg/deg)^T S
    # + b_g with v = (W_msg/deg)^-T b_g (exact for the generator's b_g=0).
    v_c = np.linalg.solve(wmc.T, bg).astype(f32)
    v_p = np.linalg.solve(wmp.T, bg).astype(f32)
    smalls = np.empty((_FV, _S4P0), dtype=f32)
    smalls[:, 0:_FV] = ws + wmc
    smalls[:, _FV:2 * _FV] = ws + wmp
    smalls[:, 2 * _FV:3 * _FV] = wmc
    smalls[:, 3 * _FV:4 * _FV] = wmp
    w1m = np.array(W1, dtype=f32, copy=True)
    w1m[_NCLIN * _FV:, :] /= f32(_NPIX)
    # Pack to SBUF layout [p, (chunk, n)]: w1p[p, c*HID+n] = w1m[c*FV+p, n].
    w1m = np.ascontiguousarray(
        w1m.reshape(_NCHUNK, _FV, _HID).transpose(1, 0, 2).reshape(_FV, -1).astype(bf))
    aux = np.zeros((_BC, _HID + 1 + _FV), dtype=f32)
    aux[:, :_HID] = np.asarray(W2, f32).reshape(1, _HID)
    aux[:, _HID] = f32(np.asarray(b2, f32).reshape(-1)[0])
    aux[:, _HID + 1:] = np.eye(_FV, dtype=f32)
    rowaux = np.empty((1, _HID + _BC), dtype=f32)
    rowaux[0, :_HID] = np.asarray(b1, f32)
    rowaux[0, _HID:] = f32(1.0)
    return smalls.astype(bf), v_c, v_p, w1m, aux, rowaux.astype(bf)


def _xt_for_core(clinical, image, k, smalls, v_c, v_p):
    import ml_dtypes

    bf = ml_dtypes.bfloat16
    sl = slice(k * _BC, (k + 1) * _BC)
    xc = np.ascontiguousarray(clinical[sl].transpose(2, 1, 0)).reshape(_FV, _CCOLS)
    xp = np.ascontiguousarray(image[sl].transpose(2, 1, 0)).reshape(_FV, _PCOLS)
    # Host-side message-passing node sums (+ the b_g fold), replicated x4
    # for the N=512 aggregate matmuls.
    s_pix = xp.reshape(_FV, _NPIX, _BC).sum(axis=1) + v_c[:, None]
    s_clin = xc.reshape(_FV, _NCLIN, _BC).sum(axis=1) + v_p[:, None]
    out = np.empty((_FV, _XTW), dtype=bf)
    out[:, :_S4P0] = smalls
    out[:, _S4P0:_S4C0] = np.tile(s_pix, (1, 4)).astype(bf)
    out[:, _S4C0:_CLIN0] = np.tile(s_clin, (1, 4)).astype(bf)
    out[:, _CLIN0:_PIX0] = xc.astype(bf)
    out[:, _PIX0:] = xp.astype(bf)
    return out


def kernel(**inputs):
    clinical = np.asarray(inputs["clinical_embeddings"], np.float32)
    image = np.asarray(inputs["image_embeddings"], np.float32)
    smalls, v_c, v_p, w1m, aux, rowaux = _host_prep(
        inputs["W_self"], inputs["W_msg"], inputs["b_g"],
        inputs["W1"], inputs["b1"], inputs["W2"], inputs["b2"],
    )

    if "nc" not in _CACHE:
        _CACHE["nc"] = _build_bass()
    nc = _CACHE["nc"]

    in_maps = [
        {
            "xt": _xt_for_core(clinical, image, k, smalls, v_c, v_p),
            "w1": w1m,
            "aux": aux,
            "rowaux": rowaux,
        }
        for k in range(_NCORES)
    ]

    from concourse.bass_utils import run_bass_kernel_spmd

    res = run_bass_kernel_spmd(
        nc, in_maps, core_ids=list(range(_NCORES)),
        trace=bool(_CACHE.get("trace", False)),
        **_CACHE.get("run_kwargs", {}),
    )
    _CACHE["last_results"] = res
    out = np.concatenate(
        [np.asarray(r["out"], np.float32).reshape(_BC, 1) for r in res.results],
        axis=0)
    return np.ascontiguousarray(out)
